# revision 43
# baseline (speedup 1.0000x reference)
"""Trainium2 Bass kernel for nn_CCELoss (calibration-histogram loss).

Sharding: data-parallel over image rows, 8 NeuronCores, 128 rows each.

Per-core layout: logits [114 = 6 pixel-groups x 19 classes, F=45056]
(group g covers core-flat pixels [g*F, (g+1)*F); tail of group 5 is pad
with logit 0 -> p = 1/19, excluded on host).

Per 4096-pixel tile (11 tiles):
  ACT  e = exp(l)                       fp32
  PE   Z stacked [48,512] PSUM: 8 accumulating fp32r matmuls, chunk q of
       512 pixels lands at partitions 6q..6q+5 (block-ones stationary)
  ACT  m = ln(Z)  [48,512]              (one cheap 512-col op)
  PE   d = I@l - bc@m into PSUM [114,2048] halves (fp32r, accumulate)
  ACT  p = exp(d) -> bf16 SBUF (PSUM-read halves, accum_out = sum p)
  folds on bf16 p (one DVE/ACT pass per threshold; with accum_out the
  tensor_scalar op1 is the REDUCE op, so DVE folds are single-op only):
    DVE  counts N_i = sum [p>t_i] (is_gt/add); min-sums sum min(p,t_i)
         (min/add) -> relu-sum = sump - minsum (no cancellation)
    ACT  relu sums for act_set(t) thresholds (Relu bias=-t, accum);
         last two tiles ACT-heavy so the DVE fold tail hides under ACT
True-class side channel: host uploads gathered true-class logit l* in
pixel-major [96, 11*256]; device computes d* = l* - m (m re-shaped by two
small DMAs per tile) and returns p* = exp(d*) in bf16; host bins it.
Host: decode folds -> conf/count hists, bin p* -> acc hist, loss formula.
"""

import numpy as np

import bass_rust
import concourse.bass as bass
from concourse import bacc
import concourse.mybir as mybir
import concourse.tile as tile
from concourse.vector_clock import ScopedClock
from concourse.bass_utils import run_bass_kernel_spmd

F32 = mybir.dt.float32
F32R = mybir.dt.float32r
BF16 = mybir.dt.bfloat16
AF = mybir.ActivationFunctionType
ALU = mybir.AluOpType

# ---------------- problem geometry (hardcoded) ----------------
C = 19
NB = 10
H, W = 1024, 2048
NCORES = 8
ROWS = H // NCORES          # 128
NPIX = ROWS * W             # 262144 valid pixels per core
G = 6
P = G * C                   # 114 partitions
TILE_F = 4096
NT = 11
F = NT * TILE_F             # 45056
NPAD = G * F - NPIX         # 8192 pad pixels (tiles 9,10 of group 5)
PAD_TILE0 = 9               # tiles 9,10 have group 5 all-pad

THR = [np.float32(i / 10.0) for i in range(10)]
PCOLS = NT * 256            # l*/p* cols: t*256 + c256

# fold slot layout per tile (20 slots)
S_SUMP = 0                  # 2 slots: sum p accum from the two exp halves
S_CNT = 2                   # +i-1 for i=1..9: counts
S_RELU = 11                 # +i-1 for i=1..9: relu sums
NSLOT = 20
ACT_RELU_EVEN = [6, 7, 8]   # ACT relu folds on even tiles
ACT_RELU_ODD = [6, 7]       # ACT relu folds on odd tiles


# fold-tiles: tile 0 is split into two 2048-col halves (earlier fold start);
# fold-tile ft=0,1 -> tile 0 halves, ft=t+1 -> tile t for t>=1.
NFT = NT + 1
FT_WIDTH = [2048, 2048] + [4096] * (NT - 1)


def act_set(ft):
    """Which relu folds run on ACT for fold-tile ft (rest are DVE min-folds).
    Tile-0 halves all-DVE (ACT is production-bound at start); last two
    tiles ACT-heavy so the DVE fold tail hides under ACT."""
    if ft <= 1:
        return []
    if ft == NFT - 1:
        return [4, 5, 6, 7, 8]
    if ft == NFT - 2:
        return [5, 6, 7, 8]
    return ACT_RELU_EVEN if (ft - 1) % 2 == 0 else ACT_RELU_ODD

MM_CHUNK = 512
NCH = TILE_F // MM_CHUNK    # 8 chunks per tile

# bf16(exp(-ln(19))): the device's pad probability after bf16 rounding
P_PAD = 0.052734375

_BUILD_CACHE = {}


def _patch_tile_drain():
    """walrus rejects drains with >1 sync wait; split the tile-exit drain."""
    if getattr(tile.TileContext, "_drain_patched", False):
        return

    def _drain_and_barrier(self, tick_clock, wait_clock):
        drain_inst = self.nc.sync.drain()
        wait_clock.add_sem_waits(
            drain_inst.ins, ScopedClock({None: tick_clock.global_clock})
        )
        si = drain_inst.ins.sync_info
        if si is not None and len(si.on_wait) > 1:
            waits = list(si.on_wait)
            ups = list(si.on_update)
            drain_inst.ins.sync_info = mybir.SyncInfo(on_wait=waits[:1], on_update=[])
            last = drain_inst
            for i in range(1, len(waits)):
                last = self.nc.sync.drain()
                last.ins.sync_info = mybir.SyncInfo(on_wait=waits[i:i + 1], on_update=[])
            if ups:
                lw = list(last.ins.sync_info.on_wait) if last.ins.sync_info else []
                last.ins.sync_info = mybir.SyncInfo(on_wait=lw, on_update=ups)
        self.nc.all_engine_barrier()
        assert self.sems is not None
        popped = self.nc._tile_sem_poison_stack.pop()
        assert popped is self._sem_poison
        self.nc.clear_and_free_semaphores(list(self.sems.allocated().values()))
        self.nc.all_engine_barrier()

    tile.TileContext._drain_and_barrier = _drain_and_barrier
    tile.TileContext._drain_patched = True


def _patch_act_tables():
    """Prefer the exp+ln+relu set so the table-load pass emits one load
    instead of thrashing between per-function canonical sets."""
    if getattr(bacc, "_act_tables_patched", False):
        return
    orig = bacc.get_activation_tables

    def filtered(arch):
        t = dict(orig(arch))
        pref = "natural_log_exp_and_others"
        if pref in t:
            # Keep positions (act_func_set_id indexes act_info.json order);
            # empty competing sets so every func resolves to `pref`.
            t = {k: (v if k == pref else type(v)()) for k, v in t.items()}
        return t

    filtered.__wrapped__ = orig
    bacc.get_activation_tables = filtered
    bacc._act_tables_patched = True


def build_nc():
    _patch_tile_drain()
    _patch_act_tables()
    nc = bacc.Bacc()

    # const APs for ACT fold biases
    for i in (3, 4, 5, 6, 7, 8):
        for v in (float(-THR[i]), float(THR[i])):
            if (F32, v) in nc.const_aps.aps:
                continue
            tns = nc.alloc_sbuf_tensor(f"const-thr-{v}", [128, 1], F32)
            nc.gpsimd.memset(tns.ap(), v)
            nc.const_aps.aps[(F32, v)] = tns.ap()
    nc.all_engine_barrier()

    lg = nc.declare_dram_parameter("lg", [C, NPIX], F32R, isOutput=False)
    zpad = nc.declare_dram_parameter("zpad", [C, TILE_F], F32R, isOutput=False)
    lstar = nc.declare_dram_parameter("lstar", [96, PCOLS], F32, isOutput=False)
    bd48 = nc.declare_dram_parameter("bd48", [P, NCH * 48], F32R, isOutput=False)
    ident = nc.declare_dram_parameter("ident", [P, P], F32R, isOutput=False)
    bcq48 = nc.declare_dram_parameter("bcq48", [48, NCH * P], F32R, isOutput=False)
    folds_out = nc.declare_dram_parameter("folds", [P, NFT * NSLOT], F32, isOutput=True)
    pstar_out = nc.declare_dram_parameter("pstar", [96, PCOLS], BF16, isOutput=True)

    with tile.TileContext(nc) as tc:
        with (
            tc.tile_pool(name="const", bufs=1) as constp,
            tc.tile_pool(name="lt", bufs=2) as lp,
            tc.tile_pool(name="et", bufs=2) as ep,
            tc.tile_pool(name="pt", bufs=4) as pp,
            tc.tile_pool(name="mt", bufs=2) as mp,
            tc.tile_pool(name="mc", bufs=2) as mcp,
            tc.tile_pool(name="lst", bufs=2) as lsp,
            tc.tile_pool(name="acc", bufs=1) as accp,
            tc.tile_pool(name="zq", bufs=1, space="PSUM") as zqp,
            tc.tile_pool(name="zqh", bufs=1, space="PSUM") as zqhp,
            tc.tile_pool(name="dq", bufs=1, space="PSUM") as dqp,
        ):
            bd_sb = constp.tile([P, NCH * 48], F32R)
            nc.gpsimd.dma_start(out=bd_sb[:], in_=bd48[:])
            id_sb = constp.tile([P, P], F32R)
            nc.gpsimd.dma_start(out=id_sb[:], in_=ident[:])
            bc_sb = constp.tile([48, NCH * P], F32R)
            nc.gpsimd.dma_start(out=bc_sb[:], in_=bcq48[:])

            foldacc = accp.tile([P, NFT * NSLOT], F32)
            dstar = accp.tile([96, PCOLS], F32)
            scr_dve = accp.tile([P, TILE_F], BF16)
            scr_act = accp.tile([P, TILE_F], BF16)

            def issue_act_folds(ft, pt, off, width):
                base_s = ft * NSLOT
                for i in act_set(ft):
                    col = foldacc[:, base_s + S_RELU + i - 1:base_s + S_RELU + i]
                    nc.scalar.activation(
                        scr_act[:, :width], pt[:, off:off + width], AF.Relu,
                        bias=-float(THR[i]), accum_out=col)

            def issue_dve_folds(ft, pt, off, width):
                base_s = ft * NSLOT
                act_relu = act_set(ft)
                for i in range(1, 10):
                    col = foldacc[:, base_s + S_CNT + i - 1:base_s + S_CNT + i]
                    nc.vector.tensor_scalar(
                        scr_dve[:, :width], pt[:, off:off + width],
                        float(THR[i]), None, ALU.is_gt, ALU.add, accum_out=col)
                for i in range(1, 10):
                    if i in act_relu:
                        continue
                    col = foldacc[:, base_s + S_RELU + i - 1:base_s + S_RELU + i]
                    nc.vector.tensor_scalar(
                        scr_dve[:, :width], pt[:, off:off + width],
                        float(THR[i]), None, ALU.min, ALU.add, accum_out=col)

            def alloc_dq():
                return dqp.tile([P, 2048], F32, name="dq")

            pending = []  # (ft, pt, off, width) whose ACT folds are pending
            for t in range(NT):
                # ---- load logits tile [114, 4096] ----
                lt = lp.tile([P, TILE_F], F32R)
                ng = G if t < PAD_TILE0 else G - 1
                nld = 2 if t == 0 else 1
                for hb in range(nld):
                    w = TILE_F // nld
                    base = lg[:, t * TILE_F + hb * w:t * TILE_F + (hb + 1) * w]
                    src3 = bass_rust.AP(
                        tensor=base.tensor, offset=base.offset,
                        ap=[[F, ng]] + list(base.ap))
                    nc.gpsimd.dma_start(
                        out=lt[0:C * ng, hb * w:(hb + 1) * w], in_=src3)
                if ng < G:
                    nc.gpsimd.dma_start(out=lt[C * 5:P, :], in_=zpad[:])

                # ---- l* tile (pixel-major [96, 256]) ----
                lst = lsp.tile([96, 256], F32)
                nc.gpsimd.dma_start(out=lst[:], in_=lstar[:, t * 256:(t + 1) * 256])

                mc = mcp.tile([96, 256], F32)
                pt = pp.tile([P, TILE_F], BF16)

                if t == 0:
                    # tile 0 split into 2048-col halves: folds start earlier
                    for hb in range(2):
                        co = hb * 2048
                        et = ep.tile([P, 2048], F32R)
                        nc.scalar.activation(
                            et[:], lt.bitcast(F32)[:, co:co + 2048], AF.Exp)
                        zqh = zqhp.tile([24, MM_CHUNK], F32)
                        for h in range(4):
                            q = 4 * hb + h
                            nc.tensor.matmul(
                                zqh[:],
                                bd_sb[:, 48 * h:48 * h + 24],
                                et[:, h * MM_CHUNK:(h + 1) * MM_CHUNK],
                                start=(h == 0), stop=(h == 3),
                            )
                        mth = mp.tile([24, MM_CHUNK], F32R)
                        nc.scalar.activation(mth[:], zqh[:], AF.Ln)
                        for j in range(2):
                            nc.gpsimd.dma_start(
                                out=mc[48 * j + 24 * hb:48 * j + 24 * hb + 24, :],
                                in_=mth.bitcast(F32)[:, j * 256:(j + 1) * 256],
                            )
                        dqh = alloc_dq()
                        for h in range(4):
                            q = 4 * hb + h
                            sl = slice(h * MM_CHUNK, (h + 1) * MM_CHUNK)
                            nc.tensor.matmul(
                                dqh[:, sl],
                                id_sb[:],
                                lt[:, q * MM_CHUNK:(q + 1) * MM_CHUNK],
                                start=True, stop=False,
                            )
                            nc.tensor.matmul(
                                dqh[:, sl],
                                bc_sb[0:24, P * h:P * (h + 1)],
                                mth[:],
                                start=False, stop=True,
                            )
                        nc.scalar.activation(
                            pt[:, co:co + 2048], dqh[:], AF.Exp,
                            accum_out=foldacc[:, hb * NSLOT:hb * NSLOT + 1])
                        issue_dve_folds(hb, pt, co, 2048)
                    nc.vector.tensor_sub(dstar[:, 0:256], lst[:], mc[:])
                    continue

                ft = t + 1
                # ---- e = exp(l), fp32 ----
                et = ep.tile([P, TILE_F], F32R)
                nc.scalar.activation(et[:], lt.bitcast(F32)[:], AF.Exp)

                # ---- Z stacked [48, 512]: 8 accumulating fp32r matmuls ----
                zq = zqp.tile([48, MM_CHUNK], F32)
                for q in range(NCH):
                    nc.tensor.matmul(
                        zq[:],
                        bd_sb[:, 48 * q:48 * (q + 1)],
                        et[:, q * MM_CHUNK:(q + 1) * MM_CHUNK],
                        start=(q == 0), stop=(q == NCH - 1),
                    )

                # ---- m = ln(Z) [48, 512] ----
                mt = mp.tile([48, MM_CHUNK], F32R)
                nc.scalar.activation(mt[:], zq[:], AF.Ln)

                # ---- lagged ACT folds fill the PE d-matmul gap ----
                if len(pending) >= 1:
                    issue_act_folds(*pending.pop(0))

                # ---- m -> pixel-major [96, 256] (2 small DMAs) ----
                for j in range(2):
                    nc.gpsimd.dma_start(
                        out=mc[48 * j:48 * (j + 1), :],
                        in_=mt.bitcast(F32)[:, j * 256:(j + 1) * 256],
                    )

                # ---- d* = l* - m (pixel-major) ----
                nc.vector.tensor_sub(
                    dstar[:, t * 256:(t + 1) * 256], lst[:], mc[:])

                # ---- d = I@l - bc@m into PSUM halves; p = exp(d) bf16 ----
                base_s = ft * NSLOT
                for Hh in range(2):
                    dq = alloc_dq()
                    for h in range(4):
                        q = 4 * Hh + h
                        sl = slice(h * MM_CHUNK, (h + 1) * MM_CHUNK)
                        nc.tensor.matmul(
                            dq[:, sl],
                            id_sb[:],
                            lt[:, q * MM_CHUNK:(q + 1) * MM_CHUNK],
                            start=True, stop=False,
                        )
                        nc.tensor.matmul(
                            dq[:, sl],
                            bc_sb[:, P * q:P * (q + 1)],
                            mt[:],
                            start=False, stop=True,
                        )
                    nc.scalar.activation(
                        pt[:, Hh * 2048:(Hh + 1) * 2048], dq[:], AF.Exp,
                        accum_out=foldacc[:, base_s + Hh:base_s + Hh + 1])

                # ---- DVE folds on bf16 p ----
                issue_dve_folds(ft, pt, 0, TILE_F)
                pending.append((ft, pt, 0, TILE_F))

            # ---- end phase: pstar first (doesn't need the last pt) ----
            pstar_sb = accp.tile([96, PCOLS], BF16)
            nc.scalar.activation(pstar_sb[:], dstar[:], AF.Exp)
            nc.gpsimd.dma_start(out=pstar_out[:], in_=pstar_sb[:])
            for pf in pending:
                issue_act_folds(*pf)
            nc.gpsimd.dma_start(out=folds_out[:], in_=foldacc[:])

    nc.finalize()
    return nc


def _make_consts():
    bd48 = np.zeros((P, NCH * 48), np.float32)
    for q in range(NCH):
        for k in range(P):
            bd48[k, 48 * q + 6 * q + k // C] = 1.0
    ident = np.eye(P, dtype=np.float32)
    bcq48 = np.zeros((48, NCH * P), np.float32)
    for q in range(NCH):
        for p in range(P):
            bcq48[6 * q + p // C, P * q + p] = -1.0
    return bd48, ident, bcq48


def _pix_major(ls_flat: np.ndarray) -> np.ndarray:
    """core-flat [G*F] -> [96, PCOLS]: partition 48j+6q+g, col t*256+c."""
    return (ls_flat.reshape(G, NT, NCH, 2, 256).transpose(3, 2, 0, 1, 4)
            .reshape(96, PCOLS))


def _pix_major_inv(ps: np.ndarray) -> np.ndarray:
    """[96, PCOLS] -> core-flat [G*F]."""
    return (ps.reshape(2, NCH, G, NT, 256).transpose(2, 3, 1, 0, 4).reshape(-1))


def _shard_host(output: np.ndarray, target: np.ndarray):
    o = np.ascontiguousarray(output[0])          # [19, 1024, 2048]
    t = np.ascontiguousarray(target[0])          # [1024, 2048]
    lstar_full = np.take_along_axis(o, t[None], axis=0)[0]
    bd48, ident, bcq48 = _make_consts()

    in_maps = []
    for core in range(NCORES):
        r0 = core * ROWS
        lgv = np.ascontiguousarray(o[:, r0:r0 + ROWS, :].reshape(C, NPIX))
        ls = lstar_full[r0:r0 + ROWS, :].reshape(-1)
        ls = np.concatenate([ls, np.zeros(NPAD, np.float32)])
        in_maps.append({
            "lg": lgv, "lstar": np.ascontiguousarray(_pix_major(ls)),
            "bd48": bd48, "ident": ident, "bcq48": bcq48,
            "zpad": np.zeros((C, TILE_F), np.float32),
        })
    return in_maps


def _decode_and_loss(results, target: np.ndarray):
    conf = np.zeros((C, NB), np.float64)
    cnt = np.zeros((C, NB), np.float64)
    acc = np.zeros((C, NB), np.float64)

    tgrid = np.arange(10, dtype=np.float64) / 10.0

    for core in range(NCORES):
        folds = results[core]["folds"].astype(np.float64)
        folds = folds.reshape(P, NFT, NSLOT)

        sump_t = folds[:, :, S_SUMP].copy()                     # [114, NFT]
        for ft in range(NFT):
            if FT_WIDTH[ft] == 4096:
                sump_t[:, ft] += folds[:, ft, S_SUMP + 1]
        N_t = folds[:, :, S_CNT:S_CNT + 9]                      # [114, NFT, 9]
        slot_t = folds[:, :, S_RELU:S_RELU + 9]                 # [114, NFT, 9]

        # per-tile effective relu sums: ACT slots hold sum relu(p-t);
        # DVE slots hold sum min(p,t): relu-sum = sump - minsum - t*(F - N)
        # ... using identity S_i = (sump - minsum) + t*N; relu = S - t*N
        # so R_eff = sump - minsum - t*... compute directly as below.
        R_eff = np.empty_like(slot_t)
        for t in range(NFT):
            act_relu = act_set(t)
            for i in range(1, 10):
                s = slot_t[:, t, i - 1]
                if i in act_relu:
                    R_eff[:, t, i - 1] = s
                else:
                    # sum min = sum_{p<=t} p + t*N_>  =>
                    # relu-sum = (sump - minsum) - t*N  ... wait:
                    # sump - minsum = S_i - t*N_i ; relu-sum = S_i - t*N_i
                    R_eff[:, t, i - 1] = sump_t[:, t] - s

        N = N_t.sum(axis=1).reshape(G, C, 9).sum(axis=0)        # [19, 9]
        R = R_eff.sum(axis=1).reshape(G, C, 9).sum(axis=0)      # [19, 9]

        # sum p over valid pixels (pad only in group-5 rows, tiles 9,10)
        sump = sump_t.sum(axis=1).reshape(G, C)
        sump[5, :] -= 2 * TILE_F * P_PAD
        sump = sump.sum(axis=0)                                 # [19]

        S = np.zeros((C, 11), np.float64)
        Ncnt = np.zeros((C, 11), np.float64)
        S[:, 1:10] = R + tgrid[1:][None, :] * N
        S[:, 0] = sump
        Ncnt[:, 1:10] = N
        Ncnt[:, 0] = float(NPIX)
        conf += S[:, :10] - S[:, 1:]
        cnt += Ncnt[:, :10] - Ncnt[:, 1:]

        # acc: bin returned bf16 p*
        r0 = core * ROWS
        ps = _pix_major_inv(
            results[core]["pstar"].astype(np.float32))[:NPIX]
        y = target[0, r0:r0 + ROWS, :].reshape(-1)
        b = np.clip(np.ceil(ps * np.float32(10.0)).astype(np.int32) - 1,
                    0, NB - 1)
        acc += np.bincount(y * NB + b, minlength=C * NB).reshape(C, NB)

    EPS = 1e-13
    avg_acc = acc / (cnt + EPS)
    avg_conf = conf / (cnt + EPS)
    loss = np.sum((avg_acc - avg_conf) ** 2 * (cnt / cnt.sum()))
    return np.float32(loss), (conf, cnt, acc)


def kernel(output: np.ndarray, target: np.ndarray) -> np.ndarray:
    output = np.asarray(output, np.float32)
    target = np.asarray(target, np.int32)
    if "nc" not in _BUILD_CACHE:
        _BUILD_CACHE["nc"] = build_nc()
    nc = _BUILD_CACHE["nc"]
    in_maps = _shard_host(output, target)
    res = run_bass_kernel_spmd(nc, in_maps, list(range(NCORES)))
    loss, _ = _decode_and_loss(res.results, target)
    return np.float32(loss)
